# revision 12
# baseline (speedup 1.0000x reference)
"""RoFormer self-attention (LN + fused QKV + interleaved RoPE + SDPA) on 8 trn2 cores.

Sharding: core c -> batch b = c//2, head-group g = c%2 (8 of 16 heads).
Each core computes LN + QKV for its batch (2x-redundant LN within a batch
pair), RoPE, and full non-causal attention for its 8 heads, writing the
[2048, 512] slice out[b, :, 512g:512(g+1)].

All matmuls run in float32r (full-rate PE, ~1e-4 rel err). Softmax skips the
max-subtraction (scores ~ N(0,1), exp is safe in fp32) and uses an appended
ones-column on V to produce the denominator on the tensor engine.
"""

import numpy as np

import concourse.bass as bass
import concourse.mybir as mybir
import concourse.tile as tile
from concourse import bacc
from concourse.bass_utils import run_bass_kernel_spmd

F32 = mybir.dt.float32
F32R = mybir.dt.float32r
AX = mybir.AluOpType
ACT = mybir.ActivationFunctionType

B, S, H = 4, 2048, 1024
NH, HD = 16, 64
LN_EPS = 1e-12
N_CORES = 8
HPC = NH // 2          # 8 heads per core
WCOLS = 3 * HPC * HD   # 1536
TOKCH = S // 128       # 16 token chunks
SCALE = 1.0 / np.sqrt(HD)

_CACHE = {}


def _bcast(ap, n, axis=1):
    """Insert a stride-0 broadcast dim of size n at `axis` of a 2D AP."""
    new = [list(p) for p in ap.ap]
    new.insert(axis, [0, n])
    return bass.AP(tensor=ap.tensor, offset=ap.offset, ap=new)


def _bias_bcast(bias_slice):
    """[1, 512] bias AP -> [128(bcast), HPC, HD] with partition stride 0."""
    return bass.AP(tensor=bias_slice.tensor, offset=bias_slice.offset,
                   ap=[[0, 128], [HD, HPC], [1, HD]])


def _build_program():
    nc = bacc.Bacc("TRN2", target_bir_lowering=False)

    hid_d = nc.dram_tensor("hid", [S, H], F32, kind="ExternalInput")
    w_d = nc.dram_tensor("w", [H, WCOLS], F32R, kind="ExternalInput")
    bias_d = nc.dram_tensor("bias", [1, WCOLS], F32, kind="ExternalInput")
    sin_d = nc.dram_tensor("sintab", [S, HD], F32, kind="ExternalInput")
    cos_d = nc.dram_tensor("costab", [S, HD], F32, kind="ExternalInput")
    id_d = nc.dram_tensor("ident", [128, 128], F32R, kind="ExternalInput")
    idf_d = nc.dram_tensor("identf", [128, 128], F32, kind="ExternalInput")
    ones_d = nc.dram_tensor("ones", [128, 1], F32R, kind="ExternalInput")
    out_d = nc.dram_tensor("out", [S, HPC * HD], F32, kind="ExternalOutput")

    with tile.TileContext(nc) as tc:
        with tc.tile_pool(name="const", bufs=1) as const, \
             tc.tile_pool(name="store", bufs=1) as store:
            bias_s = const.tile([128, WCOLS], F32)
            nc.sync.dma_start(out=bias_s, in_=bass.AP(
                tensor=bias_d, offset=0, ap=[[0, 128], [1, WCOLS]]))
            sin_s = const.tile([128, TOKCH, HD], F32)
            nc.sync.dma_start(out=sin_s, in_=sin_d.rearrange("(t p) d -> p t d", p=128))
            cos_s = const.tile([128, TOKCH, HD], F32)
            nc.sync.dma_start(out=cos_s, in_=cos_d.rearrange("(t p) d -> p t d", p=128))
            id_s = const.tile([128, 128], F32R)
            nc.sync.dma_start(out=id_s, in_=id_d[:, :])
            idf_s = const.tile([128, 128], F32)
            nc.sync.dma_start(out=idf_s, in_=idf_d[:, :])
            ones_s = const.tile([128, 1], F32R)
            nc.sync.dma_start(out=ones_s, in_=ones_d[:, :])

            # Transposed per-head q/k: head h lives at partitions (h%2)*64,
            # pair index h//2:  [128, 4, TOKCH, 128]  (= [64, tokch*128] per head)
            qT = store.tile([128, HPC // 2, TOKCH, 128], F32R)
            kT = store.tile([128, HPC // 2, TOKCH, 128], F32R)
            # v with appended ones column: [tok, head, 65]
            vA = store.tile([128, TOKCH, HPC, HD + 1], F32R)

            # ---------------- Phase 1: LN + QKV + RoPE + transposes ----------
            with tc.tile_pool(name="wpool", bufs=1) as wpool, \
                 tc.tile_pool(name="p1", bufs=2) as p1, \
                 tc.tile_pool(name="p1s", bufs=4) as p1s, \
                 tc.tile_pool(name="trp", bufs=2, space="PSUM") as trp, \
                 tc.tile_pool(name="qkvp", bufs=2, space="PSUM") as qkvp:
                w_s = wpool.tile([128, H // 128, WCOLS], F32R)
                nc.sync.dma_start(out=w_s, in_=w_d.rearrange("(a p) n -> p a n", p=128))

                for t in range(TOKCH):
                    ht = p1.tile([128, H], F32, tag="ht")
                    nc.sync.dma_start(out=ht, in_=hid_d[t * 128:(t + 1) * 128, :])

                    st6 = p1s.tile([128, 2, 6], F32, tag="st6")
                    for half in range(2):
                        nc.vector.bn_stats(out=st6[:, half, :],
                                           in_=ht[:, half * 512:(half + 1) * 512])
                    mv = p1s.tile([128, 2], F32, tag="mv")
                    nc.vector.bn_aggr(out=mv, in_=st6)
                    vpe = p1s.tile([128, 1], F32, tag="vpe")
                    nc.vector.tensor_scalar(out=vpe, in0=mv[:, 1:2], scalar1=LN_EPS,
                                            scalar2=None, op0=AX.add)
                    rv = p1s.tile([128, 1], F32, tag="rv")
                    nc.vector.reciprocal(rv, vpe)
                    rstd = p1s.tile([128, 1], F32, tag="rstd")
                    nc.scalar.activation(rstd, rv, ACT.Sqrt)

                    hn = p1.tile([128, H], F32R, tag="hn")
                    nc.vector.tensor_scalar(out=hn, in0=ht, scalar1=mv[:, 0:1],
                                            scalar2=rstd, op0=AX.subtract, op1=AX.mult)

                    # transpose hn -> hT [hch, tok]
                    hT = p1.tile([128, H // 128, 128], F32R, tag="hT")
                    for hc in range(H // 128):
                        pt = trp.tile([128, 128], F32R, tag="pt")
                        nc.tensor.transpose(pt, hn[:, hc * 128:(hc + 1) * 128], id_s)
                        nc.scalar.copy(hT[:, hc, :], pt)

                    # QKV: out[tok, n] accumulated over h-chunks
                    pq = []
                    for nch in range(3):
                        pp = qkvp.tile([128, 512], F32, tag=f"qkv{nch}")
                        for hc in range(H // 128):
                            nc.tensor.matmul(pp, lhsT=hT[:, hc, :],
                                             rhs=w_s[:, hc, nch * 512:(nch + 1) * 512],
                                             start=(hc == 0), stop=(hc == H // 128 - 1))
                        pq.append(pp)

                    # RoPE on q (nch 0) and k (nch 1), in [tok, head, d] layout
                    sin_t = sin_s[:, t, :]
                    cos_t = cos_s[:, t, :]
                    for nch in range(2):
                        p = pq[nch].rearrange("p (h d) -> p h d", d=HD)
                        bb = bias_s[:, nch * 512:(nch + 1) * 512].rearrange(
                            "p (h d) -> p h d", d=HD)
                        q0 = p1.tile([128, HPC, HD], F32, tag="q0")
                        nc.vector.tensor_tensor(out=q0, in0=p, in1=bb, op=AX.add)
                        rp = p1.tile([128, HPC, HD], F32, tag="rp")
                        nc.vector.tensor_tensor(out=rp[:, :, 0::2], in0=q0[:, :, 1::2],
                                                in1=_bcast(sin_t[:, 0::2], HPC), op=AX.mult)
                        nc.vector.tensor_tensor(out=rp[:, :, 1::2], in0=q0[:, :, 0::2],
                                                in1=_bcast(sin_t[:, 1::2], HPC), op=AX.mult)
                        qf = p1.tile([128, HPC, HD], F32R, tag="qf")
                        nc.vector.tensor_tensor(out=qf, in0=q0, in1=_bcast(cos_t, HPC),
                                                op=AX.mult)
                        nc.vector.tensor_tensor(out=qf, in0=qf, in1=rp, op=AX.add)

                        dst = qT if nch == 0 else kT
                        for h in range(HPC):
                            pt2 = trp.tile([64, 128], F32R, tag="pt")
                            nc.tensor.transpose(pt2, qf[:, h, :], id_s)
                            po = (h % 2) * 64
                            nc.vector.tensor_copy(dst[po:po + 64, h // 2, t, :], pt2)

                    # v with bias, plus ones column
                    vv = vA[:, t, :, :]
                    bbv = bias_s[:, 1024:1536].rearrange("p (h d) -> p h d", d=HD)
                    nc.vector.tensor_tensor(
                        out=vv[:, :, 0:HD],
                        in0=pq[2].rearrange("p (h d) -> p h d", d=HD),
                        in1=bbv, op=AX.add)
                    nc.vector.tensor_copy(vv[:, :, HD:HD + 1], _bcast(ones_s[:, 0:1], HPC))

            # ---------------- Phase 2: attention per (head, q-half) ----------
            with tc.tile_pool(name="p2", bufs=3) as p2, \
                 tc.tile_pool(name="p2b", bufs=2) as p2b, \
                 tc.tile_pool(name="p2s", bufs=4) as p2s, \
                 tc.tile_pool(name="stp", bufs=2, space="PSUM") as stp, \
                 tc.tile_pool(name="ctxp", bufs=1, space="PSUM") as ctxp, \
                 tc.tile_pool(name="ctxo", bufs=2, space="PSUM") as ctxo:
                QW = S // 2  # 1024 q columns per unit
                for h in range(HPC):
                    po = (h % 2) * 64
                    pr = h // 2
                    for qh in range(2):
                        cp = ctxp.tile([HD + 1, QW], F32, tag="ctx")
                        for kc in range(TOKCH):
                            sp = stp.tile([128, QW], F32, tag="st")
                            for nn in range(2):
                                nc.tensor.matmul(
                                    sp[:, nn * 512:(nn + 1) * 512],
                                    lhsT=kT[po:po + 64, pr, kc, :],
                                    rhs=qT[po:po + 64, pr, qh * 8 + nn * 4: qh * 8 + (nn + 1) * 4, :],
                                    start=True, stop=True)
                            P = p2.tile([128, QW], F32R, tag="P")
                            nc.scalar.activation(P, sp, ACT.Exp, scale=SCALE)
                            for nn in range(2):
                                nc.tensor.matmul(
                                    cp[:, nn * 512:(nn + 1) * 512],
                                    lhsT=vA[:, kc, h, :],
                                    rhs=P[:, nn * 512:(nn + 1) * 512],
                                    start=(kc == 0), stop=(kc == TOKCH - 1))
                        ctxs = p2b.tile([HD + 1, QW], F32, tag="ctxs")
                        nc.vector.tensor_copy(ctxs, cp)
                        outt = p2b.tile([128, QW // 128, HD], F32, tag="outt")
                        for tc_ in range(QW // 128):
                            co = ctxo.tile([128, HD + 1], F32, tag="co")
                            nc.tensor.transpose(co, ctxs[:, tc_ * 128:(tc_ + 1) * 128],
                                                idf_s[0:HD + 1, 0:HD + 1])
                            rec = p2s.tile([128, 1], F32, tag="rec")
                            nc.vector.reciprocal(rec, co[:, HD:HD + 1])
                            nc.vector.tensor_scalar(out=outt[:, tc_, :], in0=co[:, 0:HD],
                                                    scalar1=rec, scalar2=None, op0=AX.mult)
                        dst = out_d[qh * QW:(qh + 1) * QW, h * HD:(h + 1) * HD]
                        nc.sync.dma_start(
                            out=dst.rearrange("(c p) d -> p c d", p=128), in_=outt)

    nc.compile()
    return nc


def _host_inputs(hidden_states, sinusoidal_pos, ln_weight, ln_bias, w_qkv, b_qkv):
    """Build the per-core input maps (all numpy, fp32)."""
    hidden_states = np.ascontiguousarray(hidden_states, dtype=np.float32)
    w_qkv = np.asarray(w_qkv, dtype=np.float32)
    b_qkv = np.asarray(b_qkv, dtype=np.float32)
    ln_weight = np.asarray(ln_weight, dtype=np.float32)
    ln_bias = np.asarray(ln_bias, dtype=np.float32)
    sp = np.asarray(sinusoidal_pos, dtype=np.float32).reshape(S, HD)

    # Fold LayerNorm affine params into the projection.
    w_eff = ln_weight[:, None] * w_qkv          # [H, 3H]
    b_eff = b_qkv + ln_bias @ w_qkv             # [3H]

    sin = sp[:, :HD // 2]
    cos = sp[:, HD // 2:]
    sin_pos = np.repeat(sin, 2, axis=1)          # [S, 64], col 2i = 2i+1 = sin_i
    cos_pos = np.repeat(cos, 2, axis=1)
    sgn = np.ones((1, HD), np.float32)
    sgn[0, 0::2] = -1.0
    sin_signed = (sin_pos * sgn).astype(np.float32)  # col 2i = -sin_i, 2i+1 = sin_i

    ident = np.eye(128, dtype=np.float32)
    ones = np.ones((128, 1), np.float32)

    in_maps = []
    for c in range(N_CORES):
        b = c // 2
        g = c % 2
        cols = np.concatenate([
            np.arange(g * 512, (g + 1) * 512),
            1024 + np.arange(g * 512, (g + 1) * 512),
            2048 + np.arange(g * 512, (g + 1) * 512),
        ])
        in_maps.append({
            "hid": hidden_states[b],
            "w": np.ascontiguousarray(w_eff[:, cols]),
            "bias": np.ascontiguousarray(b_eff[cols])[None, :],
            "sintab": sin_signed,
            "costab": cos_pos,
            "ident": ident,
            "identf": ident,
            "ones": ones,
        })
    return in_maps


def _run(trace=False, **inputs):
    if "nc" not in _CACHE:
        _CACHE["nc"] = _build_program()
    nc = _CACHE["nc"]
    in_maps = _host_inputs(**inputs)
    res = run_bass_kernel_spmd(nc, in_maps, core_ids=list(range(N_CORES)),
                               trace=trace)
    out = np.empty((B, S, H), np.float32)
    for c in range(N_CORES):
        b = c // 2
        g = c % 2
        out[b, :, g * 512:(g + 1) * 512] = res.results[c]["out"]
    return out, res


def kernel(**inputs):
    out, _ = _run(trace=False, **inputs)
    return out


def kernel_traced(**inputs):
    return _run(trace=True, **inputs)


# revision 27
# speedup vs baseline: 33.6176x; 33.6176x over previous
"""RoFormer self-attention (LN + fused QKV + interleaved RoPE + SDPA) on 8 trn2 cores.

Sharding: core c -> batch b = c//2, head-group g = c%2 (8 of 16 heads).
Each core computes LN + QKV for its batch (2x-redundant LN within a batch
pair), RoPE, and full non-causal attention for its 8 heads, writing the
[2048, 512] slice out[b, :, 512g:512(g+1)].

All matmuls run in float32r (full-rate PE, ~1e-4 rel err). Softmax skips the
max-subtraction (scores ~ N(0,1), exp is safe in fp32) and uses an appended
ones-column on V to produce the denominator on the tensor engine.
"""

import numpy as np

import concourse.bass as bass
import concourse.mybir as mybir
import concourse.tile as tile
from concourse import bacc
from concourse.bass_utils import run_bass_kernel_spmd

F32 = mybir.dt.float32
F32R = mybir.dt.float32r
AX = mybir.AluOpType
ACT = mybir.ActivationFunctionType

B, S, H = 4, 2048, 1024
NH, HD = 16, 64
LN_EPS = 1e-12
N_CORES = 8
HPC = NH // 2          # 8 heads per core
WCOLS = 3 * HPC * HD   # 1536
TOKCH = S // 128       # 16 token chunks
SCALE = 1.0 / np.sqrt(HD)

_CACHE = {}


def _bcast(ap, n, axis=1):
    """Insert a stride-0 broadcast dim of size n at `axis` of a 2D AP."""
    new = [list(p) for p in ap.ap]
    new.insert(axis, [0, n])
    return bass.AP(tensor=ap.tensor, offset=ap.offset, ap=new)


def _bias_bcast(bias_slice):
    """[1, 512] bias AP -> [128(bcast), HPC, HD] with partition stride 0."""
    return bass.AP(tensor=bias_slice.tensor, offset=bias_slice.offset,
                   ap=[[0, 128], [HD, HPC], [1, HD]])


def _build_program(phases="12"):
    nc = bacc.Bacc("TRN2", target_bir_lowering=False)

    hid_d = nc.dram_tensor("hid", [S, H], F32, kind="ExternalInput")
    w_d = nc.dram_tensor("w", [H, WCOLS], F32R, kind="ExternalInput")
    bias_d = nc.dram_tensor("bias", [1, WCOLS], F32, kind="ExternalInput")
    sin_d = nc.dram_tensor("sintab", [S, HD], F32, kind="ExternalInput")
    cos_d = nc.dram_tensor("costab", [S, HD], F32, kind="ExternalInput")
    id_d = nc.dram_tensor("ident", [128, 128], F32R, kind="ExternalInput")
    idf_d = nc.dram_tensor("identf", [128, 128], F32, kind="ExternalInput")
    ones_d = nc.dram_tensor("ones", [128, 1], F32R, kind="ExternalInput")
    onesr_d = nc.dram_tensor("onesrow", [1, 128], F32R, kind="ExternalInput")
    wb_d = nc.dram_tensor("wbias", [1, WCOLS], F32R, kind="ExternalInput")
    out_d = nc.dram_tensor("out", [S, HPC * HD], F32, kind="ExternalOutput")

    with tile.TileContext(nc) as tc:
        with tc.tile_pool(name="const", bufs=1) as const, \
             tc.tile_pool(name="store", bufs=1) as store:
            sin_s = const.tile([128, TOKCH, HD], F32)
            cos_s = const.tile([128, TOKCH, HD], F32)
            id_s = const.tile([128, 128], F32R)
            idf_s = const.tile([128, 128], F32)
            ones_s = const.tile([128, 1], F32R)
            onesr_s = const.tile([1, 128], F32R)
            wb_s = const.tile([1, WCOLS], F32R)
            eps_s = const.tile([128, 1], F32)
            nc.vector.memset(eps_s, LN_EPS)

            # Transposed per-head q/k: head h lives at partitions (h%2)*64,
            # pair index h//2:  [128, 4, TOKCH, 128]  (= [64, tokch*128] per head)
            qT = store.tile([128, HPC // 2, TOKCH, 128], F32R)
            kT = store.tile([128, HPC // 2, TOKCH, 128], F32R)
            # v with appended ones column: [tok, head, 65]
            vA = store.tile([128, TOKCH, HPC, HD + 1], F32R)

            # ---------------- Phase 1: LN + QKV + RoPE + transposes ----------
            with tc.tile_pool(name="wpool", bufs=1) as wpool, \
                 tc.tile_pool(name="p1", bufs=2) as p1, \
                 tc.tile_pool(name="p1h", bufs=3) as p1h, \
                 tc.tile_pool(name="p1n", bufs=3) as p1n, \
                 tc.tile_pool(name="p1s", bufs=4) as p1s, \
                 tc.tile_pool(name="trq", bufs=3, space="PSUM") as trq, \
                 tc.tile_pool(name="qkvpA", bufs=3, space="PSUM") as qkvpA, \
                 tc.tile_pool(name="qkvpB", bufs=1, space="PSUM") as qkvpB:
                w_s = wpool.tile([128, H // 128, WCOLS], F32R)
                w_r = w_d.rearrange("(a p) n -> p a n", p=128)
                ht_tiles = {}
                for tpre in range(2):
                    ht = p1h.tile([128, H], F32, tag="ht")
                    nc.sync.dma_start(out=ht, in_=hid_d[tpre * 128:(tpre + 1) * 128, :])
                    ht_tiles[tpre] = ht
                nc.sync.dma_start(out=w_s[:, 0, :], in_=w_r[:, 0, :])
                nc.sync.dma_start(out=w_s[:, 1, :], in_=w_r[:, 1, :])
                nc.sync.dma_start(out=id_s, in_=id_d[:, :])
                nc.sync.dma_start(out=sin_s, in_=sin_d.rearrange("(t p) d -> p t d", p=128))
                nc.sync.dma_start(out=cos_s, in_=cos_d.rearrange("(t p) d -> p t d", p=128))
                nc.sync.dma_start(out=idf_s, in_=idf_d[:, :])
                nc.sync.dma_start(out=ones_s, in_=ones_d[:, :])
                nc.sync.dma_start(out=onesr_s, in_=onesr_d[:, :])
                nc.sync.dma_start(out=wb_s, in_=wb_d[:, :])

                def rope_block(t, pq):
                    sin_t = sin_s[:, t, :]
                    cos_t = cos_s[:, t, :]
                    # v with ones column (bias already folded into matmul)
                    vv = vA[:, t, :, :]
                    nc.scalar.copy(vv[:, :, 0:HD],
                                   pq[2].rearrange("p (h d) -> p h d", d=HD))
                    nc.vector.tensor_copy(vv[:, :, HD:HD + 1], _bcast(ones_s[:, 0:1], HPC))
                    for nch in range(2):
                        eng = nc.vector if nch == 0 else nc.gpsimd
                        if nch == 0:
                            q0 = pq[0].rearrange("p (h d) -> p h d", d=HD)
                        else:
                            q0 = p1.tile([128, HPC, HD], F32, tag="k0")
                            nc.scalar.copy(q0, pq[1].rearrange("p (h d) -> p h d", d=HD))
                        rp = p1.tile([128, HPC, HD], F32, tag="rp")
                        eng.tensor_tensor(out=rp[:, :, 0::2], in0=q0[:, :, 1::2],
                                          in1=_bcast(sin_t[:, 0::2], HPC), op=AX.mult)
                        eng.tensor_tensor(out=rp[:, :, 1::2], in0=q0[:, :, 0::2],
                                          in1=_bcast(sin_t[:, 1::2], HPC), op=AX.mult)
                        qf = p1.tile([128, HPC, HD], F32R, tag="qf")
                        eng.tensor_tensor(out=qf, in0=q0, in1=_bcast(cos_t, HPC),
                                          op=AX.mult)
                        eng.tensor_tensor(out=qf, in0=qf, in1=rp, op=AX.add)

                        dst = qT if nch == 0 else kT
                        pt4 = trq.tile([128, HPC // 2, 128], F32R, tag="pt4")
                        qfv = qf.rearrange("p h d -> p (h d)")
                        for j in range(HPC // 2):
                            nc.tensor.transpose(pt4[:, j, :],
                                                qfv[:, j * 128:(j + 1) * 128], id_s)
                        if nch == 0:
                            nc.vector.tensor_copy(dst[:, :, t, :], pt4)
                        else:
                            nc.scalar.copy(dst[:, :, t, :], pt4)

                pending = []  # [(t, pq)] ropes delayed 2 iterations
                for t in range(TOKCH):
                    if t + 2 < TOKCH:
                        htn = p1h.tile([128, H], F32, tag="ht")
                        nc.sync.dma_start(out=htn, in_=hid_d[(t + 2) * 128:(t + 3) * 128, :])
                        ht_tiles[t + 2] = htn
                    if t == 0:
                        for hc in range(2, H // 128):
                            nc.sync.dma_start(out=w_s[:, hc, :], in_=w_r[:, hc, :])
                    ht = ht_tiles.pop(t)

                    st6 = p1s.tile([128, 2, 6], F32, tag="st6")
                    for half in range(2):
                        nc.vector.bn_stats(out=st6[:, half, :],
                                           in_=ht[:, half * 512:(half + 1) * 512])
                    mv = p1s.tile([128, 2], F32, tag="mv")
                    nc.vector.bn_aggr(out=mv, in_=st6)
                    sd = p1s.tile([128, 1], F32, tag="sd")
                    nc.scalar.activation(sd, mv[:, 1:2], ACT.Sqrt, bias=eps_s[:, 0:1])
                    rstd = p1s.tile([128, 1], F32, tag="rstd")
                    nc.vector.reciprocal(rstd, sd)

                    hn = p1n.tile([128, H], F32R, tag="hn")
                    nc.vector.tensor_scalar(out=hn, in0=ht, scalar1=mv[:, 0:1],
                                            scalar2=rstd, op0=AX.subtract, op1=AX.mult)

                    # transpose hn -> hT [hch, tok]
                    hT = p1.tile([128, H // 128, 128], F32R, tag="hT")
                    for g in range(2):
                        ptg = trq.tile([128, 4, 128], F32R, tag="pt4")
                        for hc in range(4):
                            nc.tensor.transpose(ptg[:, hc, :],
                                                hn[:, (g * 4 + hc) * 128:(g * 4 + hc + 1) * 128], id_s)
                        nc.scalar.copy(hT[:, g * 4:(g + 1) * 4, :], ptg)

                    # QKV: out[tok, n] accumulated over h-chunks (bias via ones-row)
                    pq = []
                    for nch in range(3):
                        pool_ = qkvpA if nch == 0 else qkvpB
                        pp = pool_.tile([128, 512], F32, tag=f"qkv{nch}")
                        nc.tensor.matmul(pp, lhsT=onesr_s[:, :],
                                         rhs=wb_s[:, nch * 512:(nch + 1) * 512],
                                         start=True, stop=False)
                        for hc in range(H // 128):
                            nc.tensor.matmul(pp, lhsT=hT[:, hc, :],
                                             rhs=w_s[:, hc, nch * 512:(nch + 1) * 512],
                                             start=False, stop=(hc == H // 128 - 1))
                        pq.append(pp)

                    pending.append((t, pq))
                    if len(pending) > 2:
                        rope_block(*pending.pop(0))
                for item in pending:
                    rope_block(*item)

            # ---------------- Phase 2: attention per (head, q-half) ----------
            if "2" not in phases:
                _skip_phase2 = True
            else:
                _skip_phase2 = False
            with tc.tile_pool(name="p2", bufs=3) as p2, \
                 tc.tile_pool(name="p2b", bufs=2) as p2b, \
                 tc.tile_pool(name="p2s", bufs=4) as p2s, \
                 tc.tile_pool(name="stp", bufs=2, space="PSUM") as stp, \
                 tc.tile_pool(name="ctxp", bufs=1, space="PSUM") as ctxp, \
                 tc.tile_pool(name="ctxo", bufs=2, space="PSUM") as ctxo:
                QW = S // 2  # 1024 q columns per unit
                for h in (range(HPC) if not _skip_phase2 else []):
                    po = (h % 2) * 64
                    pr = h // 2
                    for qh in range(2):
                        cp = ctxp.tile([HD + 1, QW], F32, tag="ctx")
                        for kc in range(TOKCH):
                            sp = stp.tile([128, QW], F32, tag="st")
                            for nn in range(2):
                                nc.tensor.matmul(
                                    sp[:, nn * 512:(nn + 1) * 512],
                                    lhsT=kT[po:po + 64, pr, kc, :],
                                    rhs=qT[po:po + 64, pr, qh * 8 + nn * 4: qh * 8 + (nn + 1) * 4, :],
                                    start=True, stop=True)
                            P = p2.tile([128, QW], F32R, tag="P")
                            nc.scalar.activation(P, sp, ACT.Exp, scale=SCALE)
                            for nn in range(2):
                                nc.tensor.matmul(
                                    cp[:, nn * 512:(nn + 1) * 512],
                                    lhsT=vA[:, kc, h, :],
                                    rhs=P[:, nn * 512:(nn + 1) * 512],
                                    start=(kc == 0), stop=(kc == TOKCH - 1))
                        ctxs = p2b.tile([HD + 1, QW], F32, tag="ctxs")
                        nc.vector.tensor_copy(ctxs, cp)
                        outt = p2b.tile([128, QW // 128, HD], F32, tag="outt")
                        for tc_ in range(QW // 128):
                            co = ctxo.tile([128, HD + 1], F32, tag="co")
                            nc.tensor.transpose(co, ctxs[:, tc_ * 128:(tc_ + 1) * 128],
                                                idf_s[0:HD + 1, 0:HD + 1])
                            rec = p2s.tile([128, 1], F32, tag="rec")
                            nc.vector.reciprocal(rec, co[:, HD:HD + 1])
                            nc.vector.tensor_scalar(out=outt[:, tc_, :], in0=co[:, 0:HD],
                                                    scalar1=rec, scalar2=None, op0=AX.mult)
                        dst = out_d[qh * QW:(qh + 1) * QW, h * HD:(h + 1) * HD]
                        nc.sync.dma_start(
                            out=dst.rearrange("(c p) d -> p c d", p=128), in_=outt)

    nc.compile()
    return nc


def _host_inputs(hidden_states, sinusoidal_pos, ln_weight, ln_bias, w_qkv, b_qkv):
    """Build the per-core input maps (all numpy, fp32)."""
    hidden_states = np.ascontiguousarray(hidden_states, dtype=np.float32)
    w_qkv = np.asarray(w_qkv, dtype=np.float32)
    b_qkv = np.asarray(b_qkv, dtype=np.float32)
    ln_weight = np.asarray(ln_weight, dtype=np.float32)
    ln_bias = np.asarray(ln_bias, dtype=np.float32)
    sp = np.asarray(sinusoidal_pos, dtype=np.float32).reshape(S, HD)

    # Fold LayerNorm affine params into the projection.
    w_eff = ln_weight[:, None] * w_qkv          # [H, 3H]
    b_eff = b_qkv + ln_bias @ w_qkv             # [3H]

    sin = sp[:, :HD // 2]
    cos = sp[:, HD // 2:]
    sin_pos = np.repeat(sin, 2, axis=1)          # [S, 64], col 2i = 2i+1 = sin_i
    cos_pos = np.repeat(cos, 2, axis=1)
    sgn = np.ones((1, HD), np.float32)
    sgn[0, 0::2] = -1.0
    sin_signed = (sin_pos * sgn).astype(np.float32)  # col 2i = -sin_i, 2i+1 = sin_i

    ident = np.eye(128, dtype=np.float32)
    ones = np.ones((128, 1), np.float32)

    in_maps = []
    for c in range(N_CORES):
        b = c // 2
        g = c % 2
        cols = np.concatenate([
            np.arange(g * 512, (g + 1) * 512),
            1024 + np.arange(g * 512, (g + 1) * 512),
            2048 + np.arange(g * 512, (g + 1) * 512),
        ])
        in_maps.append({
            "hid": hidden_states[b],
            "w": np.ascontiguousarray(w_eff[:, cols]),
            "bias": np.ascontiguousarray(b_eff[cols])[None, :],
            "wbias": np.ascontiguousarray(b_eff[cols])[None, :],
            "onesrow": np.ones((1, 128), np.float32),
            "sintab": sin_signed,
            "costab": cos_pos,
            "ident": ident,
            "identf": ident,
            "ones": ones,
        })
    return in_maps


def _run(trace=False, **inputs):
    if "nc" not in _CACHE:
        _CACHE["nc"] = _build_program()
    nc = _CACHE["nc"]
    in_maps = _host_inputs(**inputs)
    res = run_bass_kernel_spmd(nc, in_maps, core_ids=list(range(N_CORES)),
                               trace=trace)
    out = np.empty((B, S, H), np.float32)
    for c in range(N_CORES):
        b = c // 2
        g = c % 2
        out[b, :, g * 512:(g + 1) * 512] = res.results[c]["out"]
    return out, res


def kernel(**inputs):
    out, _ = _run(trace=False, **inputs)
    return out


def kernel_traced(**inputs):
    return _run(trace=True, **inputs)


# revision 28
# speedup vs baseline: 33.6234x; 1.0002x over previous
"""RoFormer self-attention (LN + fused QKV + interleaved RoPE + SDPA) on 8 trn2 cores.

Sharding: core c -> batch b = c//2, head-group g = c%2 (8 of 16 heads).
Each core computes LN + QKV for its batch (2x-redundant LN within a batch
pair), RoPE, and full non-causal attention for its 8 heads, writing the
[2048, 512] slice out[b, :, 512g:512(g+1)].

All matmuls run in float32r (full-rate PE, ~1e-4 rel err). Softmax skips the
max-subtraction (scores ~ N(0,1), exp is safe in fp32) and uses an appended
ones-column on V to produce the denominator on the tensor engine.
"""

import numpy as np

import concourse.bass as bass
import concourse.mybir as mybir
import concourse.tile as tile
from concourse import bacc
from concourse.bass_utils import run_bass_kernel_spmd

F32 = mybir.dt.float32
F32R = mybir.dt.float32r
AX = mybir.AluOpType
ACT = mybir.ActivationFunctionType

B, S, H = 4, 2048, 1024
NH, HD = 16, 64
LN_EPS = 1e-12
N_CORES = 8
HPC = NH // 2          # 8 heads per core
WCOLS = 3 * HPC * HD   # 1536
TOKCH = S // 128       # 16 token chunks
SCALE = 1.0 / np.sqrt(HD)

_CACHE = {}


def _bcast(ap, n, axis=1):
    """Insert a stride-0 broadcast dim of size n at `axis` of a 2D AP."""
    new = [list(p) for p in ap.ap]
    new.insert(axis, [0, n])
    return bass.AP(tensor=ap.tensor, offset=ap.offset, ap=new)


def _bias_bcast(bias_slice):
    """[1, 512] bias AP -> [128(bcast), HPC, HD] with partition stride 0."""
    return bass.AP(tensor=bias_slice.tensor, offset=bias_slice.offset,
                   ap=[[0, 128], [HD, HPC], [1, HD]])


def _build_program(phases="12"):
    nc = bacc.Bacc("TRN2", target_bir_lowering=False)

    hid_d = nc.dram_tensor("hid", [S, H], F32, kind="ExternalInput")
    w_d = nc.dram_tensor("w", [H, WCOLS], F32R, kind="ExternalInput")
    bias_d = nc.dram_tensor("bias", [1, WCOLS], F32, kind="ExternalInput")
    sin_d = nc.dram_tensor("sintab", [S, HD], F32, kind="ExternalInput")
    cos_d = nc.dram_tensor("costab", [S, HD], F32, kind="ExternalInput")
    id_d = nc.dram_tensor("ident", [128, 128], F32R, kind="ExternalInput")
    idf_d = nc.dram_tensor("identf", [128, 128], F32, kind="ExternalInput")
    ones_d = nc.dram_tensor("ones", [128, 1], F32R, kind="ExternalInput")
    onesr_d = nc.dram_tensor("onesrow", [1, 128], F32R, kind="ExternalInput")
    wb_d = nc.dram_tensor("wbias", [1, WCOLS], F32R, kind="ExternalInput")
    out_d = nc.dram_tensor("out", [S, HPC * HD], F32, kind="ExternalOutput")

    with tile.TileContext(nc) as tc:
        with tc.tile_pool(name="const", bufs=1) as const, \
             tc.tile_pool(name="store", bufs=1) as store:
            sin_s = const.tile([128, TOKCH, HD], F32)
            cos_s = const.tile([128, TOKCH, HD], F32)
            id_s = const.tile([128, 128], F32R)
            idf_s = const.tile([128, 128], F32)
            ones_s = const.tile([128, 1], F32R)
            onesr_s = const.tile([1, 128], F32R)
            wb_s = const.tile([1, WCOLS], F32R)
            eps_s = const.tile([128, 1], F32)
            nc.vector.memset(eps_s, LN_EPS)

            # Transposed per-head q/k: head h lives at partitions (h%2)*64,
            # pair index h//2:  [128, 4, TOKCH, 128]  (= [64, tokch*128] per head)
            qT = store.tile([128, HPC // 2, TOKCH, 128], F32R)
            kT = store.tile([128, HPC // 2, TOKCH, 128], F32R)
            # v with appended ones column: [tok, head, 65]
            vA = store.tile([128, TOKCH, HPC, HD + 1], F32R)

            # ---------------- Phase 1: LN + QKV + RoPE + transposes ----------
            with tc.tile_pool(name="wpool", bufs=1) as wpool, \
                 tc.tile_pool(name="p1", bufs=2) as p1, \
                 tc.tile_pool(name="p1h", bufs=3) as p1h, \
                 tc.tile_pool(name="p1n", bufs=3) as p1n, \
                 tc.tile_pool(name="p1s", bufs=4) as p1s, \
                 tc.tile_pool(name="trq", bufs=3, space="PSUM") as trq, \
                 tc.tile_pool(name="qkvpA", bufs=3, space="PSUM") as qkvpA, \
                 tc.tile_pool(name="qkvpB", bufs=1, space="PSUM") as qkvpB:
                w_s = wpool.tile([128, H // 128, WCOLS], F32R)
                w_r = w_d.rearrange("(a p) n -> p a n", p=128)
                ht_tiles = {}
                for tpre in range(2):
                    ht = p1h.tile([128, H], F32, tag="ht")
                    nc.sync.dma_start(out=ht, in_=hid_d[tpre * 128:(tpre + 1) * 128, :])
                    ht_tiles[tpre] = ht
                nc.sync.dma_start(out=w_s[:, 0, :], in_=w_r[:, 0, :])
                nc.sync.dma_start(out=w_s[:, 1, :], in_=w_r[:, 1, :])
                nc.sync.dma_start(out=id_s, in_=id_d[:, :])
                for wu in range(24):
                    ptw = trq.tile([128, 4, 128], F32R, tag="pt4")
                    nc.tensor.transpose(ptw[:, 0, :], id_s, id_s)
                nc.sync.dma_start(out=sin_s, in_=sin_d.rearrange("(t p) d -> p t d", p=128))
                nc.sync.dma_start(out=cos_s, in_=cos_d.rearrange("(t p) d -> p t d", p=128))
                nc.sync.dma_start(out=idf_s, in_=idf_d[:, :])
                nc.sync.dma_start(out=ones_s, in_=ones_d[:, :])
                nc.sync.dma_start(out=onesr_s, in_=onesr_d[:, :])
                nc.sync.dma_start(out=wb_s, in_=wb_d[:, :])

                def rope_block(t, pq):
                    sin_t = sin_s[:, t, :]
                    cos_t = cos_s[:, t, :]
                    # v with ones column (bias already folded into matmul)
                    vv = vA[:, t, :, :]
                    nc.scalar.copy(vv[:, :, 0:HD],
                                   pq[2].rearrange("p (h d) -> p h d", d=HD))
                    nc.vector.tensor_copy(vv[:, :, HD:HD + 1], _bcast(ones_s[:, 0:1], HPC))
                    for nch in range(2):
                        eng = nc.vector if nch == 0 else nc.gpsimd
                        if nch == 0:
                            q0 = pq[0].rearrange("p (h d) -> p h d", d=HD)
                        else:
                            q0 = p1.tile([128, HPC, HD], F32, tag="k0")
                            nc.scalar.copy(q0, pq[1].rearrange("p (h d) -> p h d", d=HD))
                        rp = p1.tile([128, HPC, HD], F32, tag="rp")
                        eng.tensor_tensor(out=rp[:, :, 0::2], in0=q0[:, :, 1::2],
                                          in1=_bcast(sin_t[:, 0::2], HPC), op=AX.mult)
                        eng.tensor_tensor(out=rp[:, :, 1::2], in0=q0[:, :, 0::2],
                                          in1=_bcast(sin_t[:, 1::2], HPC), op=AX.mult)
                        qf = p1.tile([128, HPC, HD], F32R, tag="qf")
                        eng.tensor_tensor(out=qf, in0=q0, in1=_bcast(cos_t, HPC),
                                          op=AX.mult)
                        eng.tensor_tensor(out=qf, in0=qf, in1=rp, op=AX.add)

                        dst = qT if nch == 0 else kT
                        pt4 = trq.tile([128, HPC // 2, 128], F32R, tag="pt4")
                        qfv = qf.rearrange("p h d -> p (h d)")
                        for j in range(HPC // 2):
                            nc.tensor.transpose(pt4[:, j, :],
                                                qfv[:, j * 128:(j + 1) * 128], id_s)
                        if nch == 0:
                            nc.vector.tensor_copy(dst[:, :, t, :], pt4)
                        else:
                            nc.scalar.copy(dst[:, :, t, :], pt4)

                pending = []  # [(t, pq)] ropes delayed 2 iterations
                for t in range(TOKCH):
                    if t + 2 < TOKCH:
                        htn = p1h.tile([128, H], F32, tag="ht")
                        nc.sync.dma_start(out=htn, in_=hid_d[(t + 2) * 128:(t + 3) * 128, :])
                        ht_tiles[t + 2] = htn
                    if t == 0:
                        for hc in range(2, H // 128):
                            nc.sync.dma_start(out=w_s[:, hc, :], in_=w_r[:, hc, :])
                    ht = ht_tiles.pop(t)

                    st6 = p1s.tile([128, 2, 6], F32, tag="st6")
                    for half in range(2):
                        nc.vector.bn_stats(out=st6[:, half, :],
                                           in_=ht[:, half * 512:(half + 1) * 512])
                    mv = p1s.tile([128, 2], F32, tag="mv")
                    nc.vector.bn_aggr(out=mv, in_=st6)
                    sd = p1s.tile([128, 1], F32, tag="sd")
                    nc.scalar.activation(sd, mv[:, 1:2], ACT.Sqrt, bias=eps_s[:, 0:1])
                    rstd = p1s.tile([128, 1], F32, tag="rstd")
                    nc.vector.reciprocal(rstd, sd)

                    hn = p1n.tile([128, H], F32R, tag="hn")
                    nc.vector.tensor_scalar(out=hn, in0=ht, scalar1=mv[:, 0:1],
                                            scalar2=rstd, op0=AX.subtract, op1=AX.mult)

                    # transpose hn -> hT [hch, tok]
                    hT = p1.tile([128, H // 128, 128], F32R, tag="hT")
                    for g in range(2):
                        ptg = trq.tile([128, 4, 128], F32R, tag="pt4")
                        for hc in range(4):
                            nc.tensor.transpose(ptg[:, hc, :],
                                                hn[:, (g * 4 + hc) * 128:(g * 4 + hc + 1) * 128], id_s)
                        nc.scalar.copy(hT[:, g * 4:(g + 1) * 4, :], ptg)

                    # QKV: out[tok, n] accumulated over h-chunks (bias via ones-row)
                    pq = []
                    for nch in range(3):
                        pool_ = qkvpA if nch == 0 else qkvpB
                        pp = pool_.tile([128, 512], F32, tag=f"qkv{nch}")
                        nc.tensor.matmul(pp, lhsT=onesr_s[:, :],
                                         rhs=wb_s[:, nch * 512:(nch + 1) * 512],
                                         start=True, stop=False)
                        for hc in range(H // 128):
                            nc.tensor.matmul(pp, lhsT=hT[:, hc, :],
                                             rhs=w_s[:, hc, nch * 512:(nch + 1) * 512],
                                             start=False, stop=(hc == H // 128 - 1))
                        pq.append(pp)

                    pending.append((t, pq))
                    if len(pending) > 2:
                        rope_block(*pending.pop(0))
                for item in pending:
                    rope_block(*item)

            # ---------------- Phase 2: attention per (head, q-half) ----------
            if "2" not in phases:
                _skip_phase2 = True
            else:
                _skip_phase2 = False
            with tc.tile_pool(name="p2", bufs=4) as p2, \
                 tc.tile_pool(name="p2b", bufs=3) as p2b, \
                 tc.tile_pool(name="p2s", bufs=4) as p2s, \
                 tc.tile_pool(name="stp", bufs=2, space="PSUM") as stp, \
                 tc.tile_pool(name="ctxp", bufs=1, space="PSUM") as ctxp, \
                 tc.tile_pool(name="ctxo", bufs=2, space="PSUM") as ctxo:
                QW = S // 2  # 1024 q columns per unit
                for h in (range(HPC) if not _skip_phase2 else []):
                    po = (h % 2) * 64
                    pr = h // 2
                    for qh in range(2):
                        cp = ctxp.tile([HD + 1, QW], F32, tag="ctx")
                        for kc in range(TOKCH):
                            sp = stp.tile([128, QW], F32, tag="st")
                            for nn in range(2):
                                nc.tensor.matmul(
                                    sp[:, nn * 512:(nn + 1) * 512],
                                    lhsT=kT[po:po + 64, pr, kc, :],
                                    rhs=qT[po:po + 64, pr, qh * 8 + nn * 4: qh * 8 + (nn + 1) * 4, :],
                                    start=True, stop=True)
                            P = p2.tile([128, QW], F32R, tag="P")
                            nc.scalar.activation(P, sp, ACT.Exp, scale=SCALE)
                            for nn in range(2):
                                nc.tensor.matmul(
                                    cp[:, nn * 512:(nn + 1) * 512],
                                    lhsT=vA[:, kc, h, :],
                                    rhs=P[:, nn * 512:(nn + 1) * 512],
                                    start=(kc == 0), stop=(kc == TOKCH - 1))
                        ctxs = p2b.tile([HD + 1, QW], F32, tag="ctxs")
                        nc.vector.tensor_copy(ctxs, cp)
                        outt = p2b.tile([128, QW // 128, HD], F32, tag="outt")
                        for tc_ in range(QW // 128):
                            co = ctxo.tile([128, HD + 1], F32, tag="co")
                            nc.tensor.transpose(co, ctxs[:, tc_ * 128:(tc_ + 1) * 128],
                                                idf_s[0:HD + 1, 0:HD + 1])
                            rec = p2s.tile([128, 1], F32, tag="rec")
                            nc.vector.reciprocal(rec, co[:, HD:HD + 1])
                            nc.vector.tensor_scalar(out=outt[:, tc_, :], in0=co[:, 0:HD],
                                                    scalar1=rec, scalar2=None, op0=AX.mult)
                        dst = out_d[qh * QW:(qh + 1) * QW, h * HD:(h + 1) * HD]
                        nc.sync.dma_start(
                            out=dst.rearrange("(c p) d -> p c d", p=128), in_=outt)

    nc.compile()
    return nc


def _host_inputs(hidden_states, sinusoidal_pos, ln_weight, ln_bias, w_qkv, b_qkv):
    """Build the per-core input maps (all numpy, fp32)."""
    hidden_states = np.ascontiguousarray(hidden_states, dtype=np.float32)
    w_qkv = np.asarray(w_qkv, dtype=np.float32)
    b_qkv = np.asarray(b_qkv, dtype=np.float32)
    ln_weight = np.asarray(ln_weight, dtype=np.float32)
    ln_bias = np.asarray(ln_bias, dtype=np.float32)
    sp = np.asarray(sinusoidal_pos, dtype=np.float32).reshape(S, HD)

    # Fold LayerNorm affine params into the projection.
    w_eff = ln_weight[:, None] * w_qkv          # [H, 3H]
    b_eff = b_qkv + ln_bias @ w_qkv             # [3H]

    sin = sp[:, :HD // 2]
    cos = sp[:, HD // 2:]
    sin_pos = np.repeat(sin, 2, axis=1)          # [S, 64], col 2i = 2i+1 = sin_i
    cos_pos = np.repeat(cos, 2, axis=1)
    sgn = np.ones((1, HD), np.float32)
    sgn[0, 0::2] = -1.0
    sin_signed = (sin_pos * sgn).astype(np.float32)  # col 2i = -sin_i, 2i+1 = sin_i

    ident = np.eye(128, dtype=np.float32)
    ones = np.ones((128, 1), np.float32)

    in_maps = []
    for c in range(N_CORES):
        b = c // 2
        g = c % 2
        cols = np.concatenate([
            np.arange(g * 512, (g + 1) * 512),
            1024 + np.arange(g * 512, (g + 1) * 512),
            2048 + np.arange(g * 512, (g + 1) * 512),
        ])
        in_maps.append({
            "hid": hidden_states[b],
            "w": np.ascontiguousarray(w_eff[:, cols]),
            "bias": np.ascontiguousarray(b_eff[cols])[None, :],
            "wbias": np.ascontiguousarray(b_eff[cols])[None, :],
            "onesrow": np.ones((1, 128), np.float32),
            "sintab": sin_signed,
            "costab": cos_pos,
            "ident": ident,
            "identf": ident,
            "ones": ones,
        })
    return in_maps


def _run(trace=False, **inputs):
    if "nc" not in _CACHE:
        _CACHE["nc"] = _build_program()
    nc = _CACHE["nc"]
    in_maps = _host_inputs(**inputs)
    res = run_bass_kernel_spmd(nc, in_maps, core_ids=list(range(N_CORES)),
                               trace=trace)
    out = np.empty((B, S, H), np.float32)
    for c in range(N_CORES):
        b = c // 2
        g = c % 2
        out[b, :, g * 512:(g + 1) * 512] = res.results[c]["out"]
    return out, res


def kernel(**inputs):
    out, _ = _run(trace=False, **inputs)
    return out


def kernel_traced(**inputs):
    return _run(trace=True, **inputs)
